# revision 34
# baseline (speedup 1.0000x reference)
"""Two-layer GCN (AggGCNConv) on 8 Trainium2 NeuronCores via Bass/Tile.

Math (per GCNConv layer, normalize=True, self-loops weight 1):
    deg_i  = indeg(i) + 1,  dinv = deg**-0.5
    out_i  = dinv_i * ( sum_{j->i} Hs_j + Hs_i ) + b,   Hs = dinv * (x @ W)
so aggregation is a gather + segment-sum over rows of Hs.  Layer 2
aggregates T2 = dinv * relu(out1) first and applies W2 after aggregation.

v4 design (1239us -> 840us vs the v3 baseline):
  - NO AllGather for layer 1: every core computes the FULL Hs table from a
    replicated bf16 copy of x (dinv folded into x host-side), writing it in
    a prep-native linear layout ([group, slot, win, feat], 1792B runs per
    partition) so the table write is 2x cheaper than a node-major store.
    Each core's table is in its own "core order" (own 98 windows first) so
    the own-term epilogue stays SPMD-uniform; per-core gather indices
    absorb the reordering.  Gather pairs are (win 2m, 2m+1) of a slot.
  - Layer-2 AllGather split in THREE (shard windows 0-41 / 42-69 / 70-97
    into three Shared tables) so AG2a overlaps emit-1's tail and AG2b/c
    overlap layer-2 phase-A/B aggregation.  Phase A spills its per-window
    PSUM to SBUF (bf16); B accumulates into it; C adds it back.
  - 2 gather streams for layer 1 (row ranges <= 65536 so pair indices fit
    int16; stream 0 = first 24 prep groups, filled to a 768 quota from the
    overlap region, so its gathers start at ~43% of prep), 3 must-streams
    for layer 2 (one per AG piece).  Nodes are permuted host-side
    (in-degree snake over the 784 (core,window) bins) so every window has
    ~2048 in-edges and chunk caps pad ~2% (L1) / ~12% (L2).
  - SWDGE gathers: 8-chunk (1024-idx) calls -- the ucode ring limit; a
    bigger dynamic_dma_scratch_size passes the cost model but crashes the
    runtime.  Calls are issued interleaved across streams in consumption
    order so the 6-buf msgs pool self-paces the gather DMA at ~1.42ns/edge.
  - Segment-sum via one-hot matmul: S[p,m] = (drel2[p]==m) built per chunk
    by DVE tensor_scalar is_equal (bf16 2x mode, [128,256]); PE accumulates
    S.T @ msgs into a PSUM tile per 128-dst window; drel2 = dst%128 +
    128*parity selects the pair half.  Padding slots gather row 0 and are
    masked by drel=-1.
  - Fused epilogues: u = psum*dinv + own in one scalar_tensor_tensor;
    relu+scale folded into Activations (t2 table and own2 kept SBUF
    -resident in bf16); prep matmuls batch 7 windows per PSUM tile with a
    single batched psum->bf16 cast; one fused log_softmax tail (logits
    are tiny, so no max-subtraction is needed).
"""

import numpy as np

P = 128
F_IN, HID, CLS = 128, 64, 16
N_CORES = 8
CALL_CHUNKS = 8  # 1024 idx per dma_gather call
G_WIN = 14  # windows per PSUM/staging group
_EXEC_NS = None


def last_exec_ns():
    return _EXEC_NS


# ----------------------------------------------------------------------------
# host-side planning
# ----------------------------------------------------------------------------
class Plan:
    pass


def _pack2(trow, cwl, n_win, lo1, hi0, target0):
    """Split edges (of one core) into 2 streams: stream 0 = rows < hi0,
    stream 1 = rows >= lo1.  Rows in [lo1, hi0) may go either way; stream 0
    is filled up to `target0` per window.  Returns (counts[n_win,2], b)."""
    must0 = trow < lo1
    choice = (trow >= lo1) & (trow < hi0)
    b = np.ones(len(trow), dtype=np.int8)
    b[must0] = 0
    count = np.zeros((n_win, 2), dtype=np.int64)
    np.add.at(count, (cwl[must0], 0), 1)
    m = np.where(choice)[0]
    order = m[np.argsort(cwl[m], kind="stable")]
    sup = np.bincount(cwl[order], minlength=n_win)
    quota = np.clip(target0 - count[:, 0], 0, sup)
    st = np.zeros(n_win + 1, dtype=np.int64)
    np.cumsum(sup, out=st[1:])
    rank = np.arange(len(order)) - st[cwl[order]]
    tolow = rank < quota[cwl[order]]
    b[order[tolow]] = 0
    count[:, 0] += np.bincount(cwl[order[tolow]], minlength=n_win)
    count[:, 1] = np.bincount(cwl[b == 1], minlength=n_win)
    return count, b


def make_plan(src, dst, n_nodes, n_cores=N_CORES):
    pl = Plan()
    npc = 12544
    n_pad = npc * n_cores
    n_win = npc // P  # 98 windows per core
    NWG = n_win * n_cores  # 784 global windows
    assert n_nodes <= n_pad
    pl.npc, pl.n_pad, pl.n_win, pl.NWG = npc, n_pad, n_win, NWG
    pl.n_cores = n_cores

    # ---- degree-balanced node permutation (snake over 784 bins) ----------
    indeg = np.bincount(dst, minlength=n_pad).astype(np.int64)
    rank = np.argsort(-indeg, kind="stable")
    k = np.arange(n_pad)
    rnd, pos = k // NWG, k % NWG
    binid = np.where(rnd % 2 == 0, pos, NWG - 1 - pos)
    newrow_of_rank = (binid // n_win) * npc + (binid % n_win) * P + rnd
    newrow = np.empty(n_pad, dtype=np.int64)
    newrow[rank] = newrow_of_rank
    src2, dst2 = newrow[src], newrow[dst]
    pl.newrow = newrow  # old node id -> table row

    deg = (np.bincount(dst2, minlength=n_pad) + 1.0).astype(np.float32)
    dinv = deg**-0.5
    pl.dinv = dinv  # by new row

    c_of = dst2 // npc
    w_of = (dst2 % npc) // P
    drel = dst2 % P

    # ---- layer-2 table rows (global: core-major thirds) ------------------
    # AG2 is split in 3 (shard windows 0-41 / 42-69 / 70-97) so each piece
    # overlaps preceding work; t2_tab regions are rank-major per piece.
    HS1, HS2 = 42 * P, 28 * P
    R1, R2 = n_cores * HS1, n_cores * (HS1 + HS2)
    cc2, j2 = src2 // npc, src2 % npc
    t2row = np.select(
        [j2 < HS1, j2 < HS1 + HS2],
        [cc2 * HS1 + j2, R1 + cc2 * HS2 + (j2 - HS1)],
        R2 + cc2 * HS2 + (j2 - HS1 - HS2),
    )
    pl.HS1, pl.HS2, pl.R1, pl.R2 = HS1, HS2, R1, R2

    # stream address ranges (<= 65536 rows each -> pair idx fits int16)
    GR = G_WIN * P  # 1792 rows per prep group
    pl.l1_ranges = [(0, 24 * GR), (20 * GR, n_pad)]
    pl.l2_ranges = [(0, R1), (R1, R2), (R2, n_pad)]
    T0_L1 = 768

    # ---- stream assignment + per-core packing ----------------------------
    NS = 5  # streams: L1 s0/s1, L2 A/B/C
    counts = np.zeros((NS, n_cores, n_win), dtype=np.int64)
    percore = []
    for c in range(n_cores):
        m = np.where(c_of == c)[0]
        cwl = w_of[m]
        gwm = src2[m] // P
        posw = np.where(
            (gwm >= c * n_win) & (gwm < (c + 1) * n_win),
            gwm - c * n_win,
            n_win + np.where(gwm < c * n_win, gwm, gwm - n_win),
        )
        sl = src2[m] % P
        trow1 = ((posw // G_WIN) * P + sl) * G_WIN + posw % G_WIN
        cnt1, b1 = _pack2(
            trow1, cwl, n_win, pl.l1_ranges[1][0], pl.l1_ranges[0][1], T0_L1
        )
        t2r = t2row[m]
        b2 = ((t2r >= R1).astype(np.int8) + (t2r >= R2)).astype(np.int8)
        counts[0, c], counts[1, c] = cnt1[:, 0], cnt1[:, 1]
        for q in range(3):
            counts[2 + q, c] = np.bincount(cwl[b2 == q], minlength=n_win)
        percore.append((m, cwl, trow1, b1, t2r, b2))

    cap = -(-counts.max(axis=1) // P)  # [NS, n_win] chunks
    pl.cap = cap
    woff = np.zeros((NS, n_win), dtype=np.int64)
    nchunks_s = np.zeros(NS, dtype=np.int64)
    for s in range(NS):
        woff[s] = np.concatenate([[0], np.cumsum(cap[s])[:-1]])
        nchunks_s[s] = cap[s].sum()
    pl.woff, pl.nchunks_s = woff, nchunks_s
    sstart = np.concatenate([[0], np.cumsum(nchunks_s)])
    pl.sstart = sstart
    pl.total_chunks = int(sstart[-1])
    pl.total_slots = pl.total_chunks * P

    pl.groups = [
        list(range(g, min(g + G_WIN, n_win))) for g in range(0, n_win, G_WIN)
    ]

    pl.idx16, pl.drel, pl.dv, pl.dv2, pl.dvf, pl.corder = [], [], [], [], [], []
    for c in range(n_cores):
        m, cwl, trow1, b1, t2r, b2 = percore[c]
        idx_arr = np.zeros(pl.total_slots, dtype=np.int16)
        dr_arr = np.full(pl.total_slots, -1.0, dtype=np.float32)
        for s0, nsl, trow, b in [(0, 2, trow1, b1.astype(np.int64)),
                                 (2, 3, t2r, b2.astype(np.int64))]:
            rng = pl.l1_ranges if s0 == 0 else pl.l2_ranges
            roff = trow - np.array([r[0] for r in rng])[b]
            key = cwl * nsl + b
            order = np.argsort(key, kind="stable")
            cnts = np.stack(
                [counts[s0 + q, c] for q in range(nsl)], axis=1
            )
            starts = np.zeros(n_win * nsl + 1, dtype=np.int64)
            np.cumsum(cnts.reshape(-1), out=starts[1:])
            offs = np.arange(len(m)) - starts[key[order]]
            sb = sstart[s0 + b[order]] + woff[s0 + b[order], cwl[order]]
            pos = sb * P + offs
            idx_arr[pos] = (roff[order] // 2).astype(np.int16)
            dr_arr[pos] = drel[m][order] + P * (roff[order] % 2)

        blk = idx_arr.reshape(pl.total_slots // 16, 16).T
        pl.idx16.append(np.tile(blk, (8, 1)).copy())
        pl.drel.append(np.ascontiguousarray(dr_arr.reshape(pl.total_chunks, P).T))

        dvc = dinv[c * npc : (c + 1) * npc].reshape(n_win, P).T
        pl.dv.append(np.ascontiguousarray(dvc))
        pl.dv2.append(np.ascontiguousarray(dvc * dvc))
        corder = np.concatenate(
            [
                np.arange(c * n_win, (c + 1) * n_win),
                np.arange(0, c * n_win),
                np.arange((c + 1) * n_win, NWG),
            ]
        )
        pl.corder.append(corder)
        dvf = dinv.reshape(NWG, P)[corder].transpose(1, 0)
        pl.dvf.append(np.ascontiguousarray(dvf))
    return pl


# ----------------------------------------------------------------------------
# device kernel
# ----------------------------------------------------------------------------
def _call_list(pl, streams):
    """Gather calls for `streams`, interleaved in consumption order."""
    calls = []
    for s in streams:
        nch = int(pl.nchunks_s[s])
        for blk, c0 in enumerate(range(0, nch, CALL_CHUNKS)):
            ck = min(CALL_CHUNKS, nch - c0)
            w = int(np.searchsorted(pl.woff[s], c0, side="right")) - 1
            calls.append((s, blk, c0, ck, w))
    calls.sort(key=lambda t: (t[4], t[0]))
    return calls


def build_nc(pl, f_in=F_IN, hid=HID, cls_=CLS):
    import concourse.bacc as bacc
    import concourse.mybir as mybir
    import concourse.tile as tile
    from concourse.ap import AP as _AP

    fp32 = mybir.dt.float32
    bf16 = mybir.dt.bfloat16
    i16 = mybir.dt.int16
    Alu = mybir.AluOpType
    Act = mybir.ActivationFunctionType

    nc = bacc.Bacc(
        "TRN2",
        target_bir_lowering=False,
        debug=False,
        num_devices=pl.n_cores,
    )
    npc, n_win, NWG = pl.npc, pl.n_win, pl.NWG
    groups = pl.groups
    D2 = 2 * hid
    CC = CALL_CHUNKS
    NPAIR = pl.n_pad // 2

    xt_in = nc.dram_tensor("xt", [f_in, NWG, P], bf16, kind="ExternalInput")
    w1_in = nc.dram_tensor("w1", [f_in, hid], bf16, kind="ExternalInput")
    w2_in = nc.dram_tensor("w2", [hid, cls_], bf16, kind="ExternalInput")
    b1_in = nc.dram_tensor("b1r", [P, hid], fp32, kind="ExternalInput")
    b2_in = nc.dram_tensor("b2r", [P, cls_], fp32, kind="ExternalInput")
    eye_in = nc.dram_tensor("eye", [P, P], fp32, kind="ExternalInput")
    iota_in = nc.dram_tensor("iota", [P, 2 * P], bf16, kind="ExternalInput")
    idx_in = nc.dram_tensor(
        "idx", [P, pl.total_slots // 16], i16, kind="ExternalInput"
    )
    dr_in = nc.dram_tensor("dr", [P, pl.total_chunks], fp32, kind="ExternalInput")
    dv_in = nc.dram_tensor("dv", [P, n_win], fp32, kind="ExternalInput")
    dv2_in = nc.dram_tensor("dv2", [P, n_win], fp32, kind="ExternalInput")
    out_t = nc.dram_tensor("out", [P, n_win, cls_], fp32, kind="ExternalOutput")

    l1_pair = [(a // 2, b // 2) for a, b in pl.l1_ranges]

    with tile.TileContext(nc) as tc:
        with (
            tc.tile_pool(name="dram", bufs=1, space="DRAM") as dram,
            tc.tile_pool(name="const", bufs=1) as cpool,
            tc.tile_pool(name="prep", bufs=3) as prep,
            tc.tile_pool(name="prep_ps", bufs=2, space="PSUM") as prep_ps,
            tc.tile_pool(name="msgs", bufs=6) as msgs_pool,
            tc.tile_pool(name="oneh", bufs=6) as oneh,
            tc.tile_pool(name="agg_ps", bufs=3, space="PSUM") as agg_ps,
            tc.tile_pool(name="epi", bufs=4) as epi,
            tc.tile_pool(name="fin_ps", bufs=2, space="PSUM") as fin_ps,
            tc.tile_pool(name="fin2_ps", bufs=1, space="PSUM") as fin2_ps,
        ):
            # hs_tab in prep-native linear layout [group, slot, win, feat]:
            # the prep write is one contiguous 1792B run per partition, and
            # gather pairs are (win 2m, 2m+1) of the same slot.  Gathers use
            # a flat pair-major view ([NPAIR, D2] bitcast) of the storage.
            hs_tab = dram.tile([NWG // G_WIN, P, G_WIN, hid], bf16, name="hs_tab")
            t2_shard = dram.tile([n_win, P, hid], bf16, name="t2_shard")
            t2_tabA = dram.tile(
                [pl.R1 // 2, D2], bf16, addr_space="Shared", name="t2_tabA"
            )
            t2_tabB = dram.tile(
                [(pl.R2 - pl.R1) // 2, D2], bf16, addr_space="Shared",
                name="t2_tabB",
            )
            t2_tabC = dram.tile(
                [(pl.n_pad - pl.R2) // 2, D2], bf16, addr_space="Shared",
                name="t2_tabC",
            )

            w1_sb = cpool.tile([f_in, hid], bf16)
            nc.sync.dma_start(w1_sb[:], w1_in[:])
            w2_sb = cpool.tile([hid, cls_], bf16)
            nc.sync.dma_start(w2_sb[:], w2_in[:])
            b1_sb = cpool.tile([P, hid], fp32)
            nc.sync.dma_start(b1_sb[:], b1_in[:])
            b2_sb = cpool.tile([P, cls_], fp32)
            nc.sync.dma_start(b2_sb[:], b2_in[:])
            eye_sb = cpool.tile([P, P], fp32)
            nc.sync.dma_start(eye_sb[:], eye_in[:])
            iota_sb = cpool.tile([P, 2 * P], bf16)
            nc.sync.dma_start(iota_sb[:], iota_in[:])
            dv_sb = cpool.tile([P, n_win], fp32)
            nc.sync.dma_start(dv_sb[:], dv_in[:])
            dv2_sb = cpool.tile([P, n_win], fp32)
            nc.sync.dma_start(dv2_sb[:], dv2_in[:])
            idx_sb = cpool.tile([P, pl.total_slots // 16], i16)
            nc.sync.dma_start(idx_sb[:], idx_in[:])
            dr_sb = cpool.tile([P, pl.total_chunks], fp32)
            nc.sync.dma_start(dr_sb[:], dr_in[:])
            own1_all = cpool.tile([P, n_win, hid], bf16)
            own2_all = cpool.tile([P, n_win, hid], bf16)
            accA = cpool.tile([P, n_win, hid], bf16)
            obr_all = cpool.tile([P, n_win, cls_], fp32)
            se_all = cpool.tile([P, n_win], fp32)
            ls_all = cpool.tile([P, n_win], fp32)

            # ---- prep: replicated full table (core order) + own terms ----
            # dinv is folded into x host-side, so the table cast is a plain
            # batched psum->bf16 copy and own1 = dinv*Hs + b1.
            PB = 7  # windows per psum tile / cast op
            for g0 in range(0, NWG, G_WIN):
                gn = min(G_WIN, NWG - g0)
                xT = prep.tile([P, gn, P], bf16, tag="xT")
                nc.sync.dma_start(xT[:], xt_in[:, g0 : g0 + gn, :])
                hsd = prep.tile([P, gn, hid], bf16, tag="hsd")
                for k0 in range(0, gn, PB):
                    kb = min(PB, gn - k0)
                    ph4 = prep_ps.tile([P, PB, hid], fp32, tag="ph")
                    for j in range(kb):
                        w = g0 + k0 + j
                        nc.tensor.matmul(
                            ph4[:, j, :],
                            xT[:, k0 + j, :],
                            w1_sb[:],
                            start=True,
                            stop=True,
                        )
                        if w < n_win:  # own window (core order => first 98)
                            nc.vector.scalar_tensor_tensor(
                                own1_all[:, w, :],
                                ph4[:, j, :],
                                dv_sb[:, w : w + 1],
                                b1_sb[:],
                                Alu.mult,
                                Alu.add,
                            )
                    nc.scalar.activation(
                        hsd[:, k0 : k0 + kb, :], ph4[:, :kb, :], Act.Copy
                    )
                nc.sync.dma_start(hs_tab[g0 // G_WIN, :, :, :], hsd[:])

            def hs_view(s):
                pa, pb = l1_pair[s]
                return _AP(
                    hs_tab[:, :, :, :].tensor, pa * D2, [[D2, pb - pa], [1, D2]]
                )

            def gathers(streams, views):
                mt = {}
                for s, blk, c0, ck, _w in _call_list(pl, streams):
                    m = msgs_pool.tile([P, ck, D2], bf16, tag="m")
                    gc0 = int(pl.sstart[s]) + c0
                    nc.gpsimd.dma_gather(
                        m[:],
                        views[s - streams[0]],
                        idx_sb[:, gc0 * 8 : (gc0 + ck) * 8],
                        ck * P,
                        ck * P,
                        D2,
                    )
                    mt[(s, blk)] = m
                return mt

            def window_agg(pw, w, streams, mt, preloaded=False):
                nmm = 1 if preloaded else 0
                tot = 2 * int(sum(pl.cap[s][w] for s in streams)) + nmm
                for s in streams:
                    capw = int(pl.cap[s][w])
                    for k in range(capw):
                        pos = int(pl.woff[s][w]) + k
                        col = int(pl.sstart[s]) + pos
                        blk, off = divmod(pos, CC)
                        S = oneh.tile([P, 2 * P], bf16, tag="S")
                        nc.vector.tensor_scalar(
                            S[:],
                            iota_sb[:],
                            dr_sb[:, col : col + 1],
                            None,
                            Alu.is_equal,
                        )
                        for half in range(2):
                            nc.tensor.matmul(
                                pw[:],
                                S[:, half * P : (half + 1) * P],
                                mt[(s, blk)][
                                    :, off, half * hid : (half + 1) * hid
                                ],
                                start=(nmm == 0),
                                stop=(nmm == tot - 1),
                            )
                            nmm += 1

            # ---- layer 1 emit --------------------------------------------
            mt1 = gathers([0, 1], [hs_view(0), hs_view(1)])
            NW1, NW2 = pl.HS1 // P, pl.HS2 // P  # 42, 28
            ag_pieces = {
                2: (t2_shard[0:NW1, :, :], t2_tabA),
                4: (t2_shard[NW1 : NW1 + NW2, :, :], t2_tabB),
                6: (t2_shard[NW1 + NW2 : n_win, :, :], t2_tabC),
            }
            for gi, ws in enumerate(groups):
                g0, gn = ws[0], len(ws)
                t2g = epi.tile([P, gn, hid], bf16, tag="t2g")
                for wi, w in enumerate(ws):
                    pw = agg_ps.tile([P, hid], fp32, tag="agg")
                    window_agg(pw, w, [0, 1], mt1)
                    u = epi.tile([P, hid], fp32, tag="u")
                    nc.vector.scalar_tensor_tensor(
                        u[:],
                        pw[:],
                        dv_sb[:, w : w + 1],
                        own1_all[:, w, :],
                        Alu.mult,
                        Alu.add,
                    )
                    nc.scalar.activation(
                        t2g[:, wi, :], u[:], Act.Relu, scale=dv_sb[:, w : w + 1]
                    )
                    nc.scalar.activation(
                        own2_all[:, w, :], u[:], Act.Relu,
                        scale=dv2_sb[:, w : w + 1],
                    )
                nc.sync.dma_start(
                    t2_shard[g0 : g0 + gn, :, :].transpose([1, 0, 2]), t2g[:]
                )
                if gi in ag_pieces:
                    shard_in, tab_out = ag_pieces[gi]
                    nc.gpsimd.collective_compute(
                        "AllGather",
                        Alu.bypass,
                        replica_groups=[list(range(pl.n_cores))],
                        ins=[shard_in.opt()],
                        outs=[tab_out[:, :].opt()],
                    )

            # ---- layer 2 phase A (after AG2a) ----------------------------
            mtA = gathers([2], [t2_tabA[:, :]])
            for w in range(n_win):
                pw = agg_ps.tile([P, hid], fp32, tag="agg")
                window_agg(pw, w, [2], mtA)
                nc.scalar.activation(accA[:, w, :], pw[:], Act.Copy)

            # ---- layer 2 phase B (after AG2b): accumulate into accA ------
            mtB = gathers([3], [t2_tabB[:, :]])
            for w in range(n_win):
                pw = agg_ps.tile([P, hid], fp32, tag="agg")
                window_agg(pw, w, [3], mtB)
                nc.vector.tensor_tensor(
                    out=accA[:, w, :], in0=pw[:], in1=accA[:, w, :], op=Alu.add
                )

            # ---- layer 2 phase C + head ----------------------------------
            mtC = gathers([4], [t2_tabC[:, :]])
            for w in range(n_win):
                pw = agg_ps.tile([P, hid], fp32, tag="agg")
                nc.scalar.activation(pw[:], accA[:, w, :], Act.Copy)
                window_agg(pw, w, [4], mtC, preloaded=True)
                u = epi.tile([P, hid], fp32, tag="u")
                nc.vector.scalar_tensor_tensor(
                    u[:],
                    pw[:],
                    dv_sb[:, w : w + 1],
                    own2_all[:, w, :],
                    Alu.mult,
                    Alu.add,
                )
                ztp = fin_ps.tile([hid, P], fp32, tag="ztp")
                nc.tensor.transpose(ztp[:], u[:], eye_sb[:])
                zt = epi.tile([hid, P], bf16, tag="zt")
                nc.scalar.activation(zt[:], ztp[:], Act.Copy)
                ops = fin2_ps.tile([P, cls_], fp32, tag="ops")
                nc.tensor.matmul(ops[:], zt[:], w2_sb[:], start=True, stop=True)
                ob = epi.tile([P, cls_], fp32, tag="ob")
                nc.vector.tensor_tensor(
                    out=ob[:], in0=ops[:], in1=b2_sb[:], op=Alu.add
                )
                obr = obr_all[:, w, :]
                nc.scalar.activation(obr, ob[:], Act.Relu)
                ex = epi.tile([P, cls_], fp32, tag="ex")
                nc.scalar.activation(
                    ex[:], obr, Act.Exp, accum_out=se_all[:, w : w + 1]
                )

            # log_softmax tail (logits tiny; no max-subtraction needed)
            nc.scalar.activation(ls_all[:], se_all[:], Act.Ln)
            ostage = cpool.tile([P, n_win, cls_], fp32)
            nc.vector.tensor_tensor(
                out=ostage[:],
                in0=obr_all[:],
                in1=ls_all[:].unsqueeze(2).to_broadcast([P, n_win, cls_]),
                op=Alu.subtract,
            )
            nc.sync.dma_start(out_t[:], ostage[:])

    nc.compile()
    return nc


# ----------------------------------------------------------------------------
# inputs / entry point
# ----------------------------------------------------------------------------
def make_in_maps(pl, x, W1, b1, W2, b2, f_in=F_IN):
    import ml_dtypes

    bf16 = ml_dtypes.bfloat16
    n = x.shape[0]
    x2 = np.zeros((pl.n_pad, f_in), dtype=np.float32)
    x2[pl.newrow[:n]] = np.asarray(x, dtype=np.float32)
    x2 *= pl.dinv[:, None]  # fold dinv into x: Hs = (dinv*x) @ W1
    x2w = x2.reshape(pl.NWG, P, f_in)
    shared = {
        "w1": np.ascontiguousarray(W1).astype(bf16),
        "w2": np.ascontiguousarray(W2).astype(bf16),
        "b1r": np.tile(np.asarray(b1, dtype=np.float32), (P, 1)),
        "b2r": np.tile(np.asarray(b2, dtype=np.float32), (P, 1)),
        "eye": np.eye(P, dtype=np.float32),
        "iota": np.tile(np.arange(2 * P, dtype=np.float32), (P, 1)).astype(
            bf16
        ),
    }
    maps = []
    for c in range(pl.n_cores):
        xt = np.ascontiguousarray(x2w[pl.corder[c]].transpose(2, 0, 1)).astype(
            bf16
        )
        maps.append(
            dict(
                shared,
                xt=xt,
                idx=pl.idx16[c],
                dr=pl.drel[c],
                dv=pl.dv[c],
                dv2=pl.dv2[c],
            )
        )
    return maps


_LAST_NC = None


def kernel(x, edge_index, W1, b1, W2, b2):
    global _EXEC_NS, _LAST_NC
    from concourse.bass_utils import run_bass_kernel_spmd

    x = np.asarray(x)
    src = np.asarray(edge_index[0]).astype(np.int64)
    dst = np.asarray(edge_index[1]).astype(np.int64)
    n = x.shape[0]

    pl = make_plan(src, dst, n)
    nc = build_nc(pl)
    _LAST_NC = nc
    in_maps = make_in_maps(pl, x, W1, b1, W2, b2)

    res = run_bass_kernel_spmd(nc, in_maps, core_ids=list(range(pl.n_cores)))
    _EXEC_NS = res.exec_time_ns
    full = np.empty((pl.n_pad, CLS), dtype=np.float32)
    for c in range(pl.n_cores):
        o = np.asarray(res.results[c]["out"])  # [P, n_win, CLS]
        full[c * pl.npc : (c + 1) * pl.npc] = o.transpose(1, 0, 2).reshape(
            pl.npc, CLS
        )
    return full[pl.newrow[:n]].astype(np.float32)


# revision 38
# speedup vs baseline: 1.0080x; 1.0080x over previous
"""Two-layer GCN (AggGCNConv) on 8 Trainium2 NeuronCores via Bass/Tile.

Math (per GCNConv layer, normalize=True, self-loops weight 1):
    deg_i  = indeg(i) + 1,  dinv = deg**-0.5
    out_i  = dinv_i * ( sum_{j->i} Hs_j + Hs_i ) + b,   Hs = dinv * (x @ W)
so aggregation is a gather + segment-sum over rows of Hs.  Layer 2
aggregates T2 = dinv * relu(out1) first and applies W2 after aggregation.

v4 design (1239us -> 840us vs the v3 baseline):
  - NO AllGather for layer 1: every core computes the FULL Hs table from a
    replicated bf16 copy of x (dinv folded into x host-side), writing it in
    a prep-native linear layout ([group, slot, win, feat], 1792B runs per
    partition) so the table write is 2x cheaper than a node-major store.
    Each core's table is in its own "core order" (own 98 windows first) so
    the own-term epilogue stays SPMD-uniform; per-core gather indices
    absorb the reordering.  Gather pairs are (win 2m, 2m+1) of a slot.
  - Layer-2 AllGather split in THREE (shard windows 0-41 / 42-69 / 70-97
    into three Shared tables) so AG2a overlaps emit-1's tail and AG2b/c
    overlap layer-2 phase-A/B aggregation.  Phase A spills its per-window
    PSUM to SBUF (bf16); B accumulates into it; C adds it back.
  - 2 gather streams for layer 1 (row ranges <= 65536 so pair indices fit
    int16; stream 0 = first 24 prep groups, filled to a 768 quota from the
    overlap region, so its gathers start at ~43% of prep), 3 must-streams
    for layer 2 (one per AG piece).  Nodes are permuted host-side
    (in-degree snake over the 784 (core,window) bins) so every window has
    ~2048 in-edges and chunk caps pad ~2% (L1) / ~12% (L2).
  - SWDGE gathers: 8-chunk (1024-idx) calls -- the ucode ring limit; a
    bigger dynamic_dma_scratch_size passes the cost model but crashes the
    runtime.  Calls are issued interleaved across streams in consumption
    order so the 6-buf msgs pool self-paces the gather DMA at ~1.42ns/edge.
  - Segment-sum via one-hot matmul: S[p,m] = (drel2[p]==m) built per chunk
    by DVE tensor_scalar is_equal (bf16 2x mode, [128,256]); PE accumulates
    S.T @ msgs into a PSUM tile per 128-dst window; drel2 = dst%128 +
    128*parity selects the pair half.  Padding slots gather row 0 and are
    masked by drel=-1.
  - Fused epilogues: u = psum*dinv + own in one scalar_tensor_tensor;
    relu+scale folded into Activations (t2 table and own2 kept SBUF
    -resident in bf16); prep matmuls batch 7 windows per PSUM tile with a
    single batched psum->bf16 cast; one fused log_softmax tail (logits
    are tiny, so no max-subtraction is needed).
"""

import numpy as np

P = 128
F_IN, HID, CLS = 128, 64, 16
N_CORES = 8
CALL_CHUNKS = 8  # 1024 idx per dma_gather call
G_WIN = 14  # windows per PSUM/staging group
_EXEC_NS = None


def last_exec_ns():
    return _EXEC_NS


# ----------------------------------------------------------------------------
# host-side planning
# ----------------------------------------------------------------------------
class Plan:
    pass


def _pack2(trow, cwl, n_win, lo1, hi0, target0):
    """Split edges (of one core) into 2 streams: stream 0 = rows < hi0,
    stream 1 = rows >= lo1.  Rows in [lo1, hi0) may go either way; stream 0
    is filled up to `target0` per window.  Returns (counts[n_win,2], b)."""
    must0 = trow < lo1
    choice = (trow >= lo1) & (trow < hi0)
    b = np.ones(len(trow), dtype=np.int8)
    b[must0] = 0
    count = np.zeros((n_win, 2), dtype=np.int64)
    np.add.at(count, (cwl[must0], 0), 1)
    m = np.where(choice)[0]
    order = m[np.argsort(cwl[m], kind="stable")]
    sup = np.bincount(cwl[order], minlength=n_win)
    quota = np.clip(target0 - count[:, 0], 0, sup)
    st = np.zeros(n_win + 1, dtype=np.int64)
    np.cumsum(sup, out=st[1:])
    rank = np.arange(len(order)) - st[cwl[order]]
    tolow = rank < quota[cwl[order]]
    b[order[tolow]] = 0
    count[:, 0] += np.bincount(cwl[order[tolow]], minlength=n_win)
    count[:, 1] = np.bincount(cwl[b == 1], minlength=n_win)
    return count, b


def make_plan(src, dst, n_nodes, n_cores=N_CORES):
    pl = Plan()
    npc = 12544
    n_pad = npc * n_cores
    n_win = npc // P  # 98 windows per core
    NWG = n_win * n_cores  # 784 global windows
    assert n_nodes <= n_pad
    pl.npc, pl.n_pad, pl.n_win, pl.NWG = npc, n_pad, n_win, NWG
    pl.n_cores = n_cores

    # ---- degree-balanced node permutation (snake over 784 bins) ----------
    indeg = np.bincount(dst, minlength=n_pad).astype(np.int64)
    rank = np.argsort(-indeg, kind="stable")
    k = np.arange(n_pad)
    rnd, pos = k // NWG, k % NWG
    binid = np.where(rnd % 2 == 0, pos, NWG - 1 - pos)
    newrow_of_rank = (binid // n_win) * npc + (binid % n_win) * P + rnd
    newrow = np.empty(n_pad, dtype=np.int64)
    newrow[rank] = newrow_of_rank
    src2, dst2 = newrow[src], newrow[dst]
    pl.newrow = newrow  # old node id -> table row

    deg = (np.bincount(dst2, minlength=n_pad) + 1.0).astype(np.float32)
    dinv = deg**-0.5
    pl.dinv = dinv  # by new row

    c_of = dst2 // npc
    w_of = (dst2 % npc) // P
    drel = dst2 % P

    # ---- layer-2 table rows (global: core-major thirds) ------------------
    # AG2 is split in 3 (shard windows 0-41 / 42-69 / 70-97) so each piece
    # overlaps preceding work; t2_tab regions are rank-major per piece.
    HS1, HS2 = 42 * P, 28 * P
    R1, R2 = n_cores * HS1, n_cores * (HS1 + HS2)
    cc2, j2 = src2 // npc, src2 % npc
    t2row = np.select(
        [j2 < HS1, j2 < HS1 + HS2],
        [cc2 * HS1 + j2, R1 + cc2 * HS2 + (j2 - HS1)],
        R2 + cc2 * HS2 + (j2 - HS1 - HS2),
    )
    pl.HS1, pl.HS2, pl.R1, pl.R2 = HS1, HS2, R1, R2

    # stream address ranges (<= 65536 rows each -> pair idx fits int16)
    GR = G_WIN * P  # 1792 rows per prep group
    pl.l1_ranges = [(0, 24 * GR), (20 * GR, n_pad)]
    pl.l2_ranges = [(0, R1), (R1, R2), (R2, n_pad)]
    T0_L1 = 768

    # ---- stream assignment + per-core packing ----------------------------
    NS = 5  # streams: L1 s0/s1, L2 A/B/C
    counts = np.zeros((NS, n_cores, n_win), dtype=np.int64)
    percore = []
    for c in range(n_cores):
        m = np.where(c_of == c)[0]
        cwl = w_of[m]
        gwm = src2[m] // P
        posw = np.where(
            (gwm >= c * n_win) & (gwm < (c + 1) * n_win),
            gwm - c * n_win,
            n_win + np.where(gwm < c * n_win, gwm, gwm - n_win),
        )
        sl = src2[m] % P
        trow1 = ((posw // G_WIN) * P + sl) * G_WIN + posw % G_WIN
        cnt1, b1 = _pack2(
            trow1, cwl, n_win, pl.l1_ranges[1][0], pl.l1_ranges[0][1], T0_L1
        )
        t2r = t2row[m]
        b2 = ((t2r >= R1).astype(np.int8) + (t2r >= R2)).astype(np.int8)
        counts[0, c], counts[1, c] = cnt1[:, 0], cnt1[:, 1]
        for q in range(3):
            counts[2 + q, c] = np.bincount(cwl[b2 == q], minlength=n_win)
        percore.append((m, cwl, trow1, b1, t2r, b2))

    cap = -(-counts.max(axis=1) // P)  # [NS, n_win] chunks
    pl.cap = cap
    woff = np.zeros((NS, n_win), dtype=np.int64)
    nchunks_s = np.zeros(NS, dtype=np.int64)
    for s in range(NS):
        woff[s] = np.concatenate([[0], np.cumsum(cap[s])[:-1]])
        nchunks_s[s] = cap[s].sum()
    pl.woff, pl.nchunks_s = woff, nchunks_s
    sstart = np.concatenate([[0], np.cumsum(nchunks_s)])
    pl.sstart = sstart
    pl.total_chunks = int(sstart[-1])
    pl.total_slots = pl.total_chunks * P

    pl.groups = [
        list(range(g, min(g + G_WIN, n_win))) for g in range(0, n_win, G_WIN)
    ]

    pl.idx16, pl.drel, pl.dv, pl.dv2, pl.dvf, pl.corder = [], [], [], [], [], []
    for c in range(n_cores):
        m, cwl, trow1, b1, t2r, b2 = percore[c]
        idx_arr = np.zeros(pl.total_slots, dtype=np.int16)
        dr_arr = np.full(pl.total_slots, -1.0, dtype=np.float32)
        for s0, nsl, trow, b in [(0, 2, trow1, b1.astype(np.int64)),
                                 (2, 3, t2r, b2.astype(np.int64))]:
            rng = pl.l1_ranges if s0 == 0 else pl.l2_ranges
            roff = trow - np.array([r[0] for r in rng])[b]
            key = cwl * nsl + b
            order = np.argsort(key, kind="stable")
            cnts = np.stack(
                [counts[s0 + q, c] for q in range(nsl)], axis=1
            )
            starts = np.zeros(n_win * nsl + 1, dtype=np.int64)
            np.cumsum(cnts.reshape(-1), out=starts[1:])
            offs = np.arange(len(m)) - starts[key[order]]
            sb = sstart[s0 + b[order]] + woff[s0 + b[order], cwl[order]]
            pos = sb * P + offs
            idx_arr[pos] = (roff[order] // 2).astype(np.int16)
            dr_arr[pos] = drel[m][order] + P * (roff[order] % 2)

        blk = idx_arr.reshape(pl.total_slots // 16, 16).T
        pl.idx16.append(np.tile(blk, (8, 1)).copy())
        pl.drel.append(np.ascontiguousarray(dr_arr.reshape(pl.total_chunks, P).T))

        dvc = dinv[c * npc : (c + 1) * npc].reshape(n_win, P).T
        pl.dv.append(np.ascontiguousarray(dvc))
        pl.dv2.append(np.ascontiguousarray(dvc * dvc))
        corder = np.concatenate(
            [
                np.arange(c * n_win, (c + 1) * n_win),
                np.arange(0, c * n_win),
                np.arange((c + 1) * n_win, NWG),
            ]
        )
        pl.corder.append(corder)
        dvf = dinv.reshape(NWG, P)[corder].transpose(1, 0)
        pl.dvf.append(np.ascontiguousarray(dvf))
    return pl


# ----------------------------------------------------------------------------
# device kernel
# ----------------------------------------------------------------------------
def _call_list(pl, streams):
    """Gather calls for `streams`, interleaved in consumption order."""
    calls = []
    for s in streams:
        nch = int(pl.nchunks_s[s])
        for blk, c0 in enumerate(range(0, nch, CALL_CHUNKS)):
            ck = min(CALL_CHUNKS, nch - c0)
            w = int(np.searchsorted(pl.woff[s], c0, side="right")) - 1
            calls.append((s, blk, c0, ck, w))
    calls.sort(key=lambda t: (t[4], t[0]))
    return calls


def build_nc(pl, f_in=F_IN, hid=HID, cls_=CLS):
    import concourse.bacc as bacc
    import concourse.mybir as mybir
    import concourse.tile as tile
    from concourse.ap import AP as _AP

    fp32 = mybir.dt.float32
    bf16 = mybir.dt.bfloat16
    i16 = mybir.dt.int16
    Alu = mybir.AluOpType
    Act = mybir.ActivationFunctionType

    nc = bacc.Bacc(
        "TRN2",
        target_bir_lowering=False,
        debug=False,
        num_devices=pl.n_cores,
    )
    npc, n_win, NWG = pl.npc, pl.n_win, pl.NWG
    groups = pl.groups
    D2 = 2 * hid
    CC = CALL_CHUNKS
    NPAIR = pl.n_pad // 2

    xt_in = nc.dram_tensor("xt", [f_in, NWG, P], bf16, kind="ExternalInput")
    w1_in = nc.dram_tensor("w1", [f_in, hid], bf16, kind="ExternalInput")
    w2_in = nc.dram_tensor("w2", [hid, cls_], bf16, kind="ExternalInput")
    b1_in = nc.dram_tensor("b1r", [P, hid], fp32, kind="ExternalInput")
    b2_in = nc.dram_tensor("b2r", [P, cls_], fp32, kind="ExternalInput")
    eye_in = nc.dram_tensor("eye", [P, P], fp32, kind="ExternalInput")
    iota_in = nc.dram_tensor("iota", [P, 2 * P], bf16, kind="ExternalInput")
    idx_in = nc.dram_tensor(
        "idx", [P, pl.total_slots // 16], i16, kind="ExternalInput"
    )
    dr_in = nc.dram_tensor("dr", [P, pl.total_chunks], fp32, kind="ExternalInput")
    dv_in = nc.dram_tensor("dv", [P, n_win], fp32, kind="ExternalInput")
    dv2_in = nc.dram_tensor("dv2", [P, n_win], fp32, kind="ExternalInput")
    out_t = nc.dram_tensor("out", [P, n_win, cls_], fp32, kind="ExternalOutput")

    l1_pair = [(a // 2, b // 2) for a, b in pl.l1_ranges]

    with tile.TileContext(nc) as tc:
        with (
            tc.tile_pool(name="dram", bufs=1, space="DRAM") as dram,
            tc.tile_pool(name="const", bufs=1) as cpool,
            tc.tile_pool(name="prep", bufs=3) as prep,
            tc.tile_pool(name="prep_ps", bufs=2, space="PSUM") as prep_ps,
            tc.tile_pool(name="msgs", bufs=6) as msgs_pool,
            tc.tile_pool(name="oneh", bufs=6) as oneh,
            tc.tile_pool(name="agg_ps", bufs=4, space="PSUM") as agg_ps,
            tc.tile_pool(name="epi", bufs=4) as epi,
            tc.tile_pool(name="fin_ps", bufs=1, space="PSUM") as fin_ps,
            tc.tile_pool(name="fin2_ps", bufs=1, space="PSUM") as fin2_ps,
        ):
            # hs_tab in prep-native linear layout [group, slot, win, feat]:
            # the prep write is one contiguous 1792B run per partition, and
            # gather pairs are (win 2m, 2m+1) of the same slot.  Gathers use
            # a flat pair-major view ([NPAIR, D2] bitcast) of the storage.
            hs_tab = dram.tile([NWG // G_WIN, P, G_WIN, hid], bf16, name="hs_tab")
            t2_shard = dram.tile([n_win, P, hid], bf16, name="t2_shard")
            t2_tabA = dram.tile(
                [pl.R1 // 2, D2], bf16, addr_space="Shared", name="t2_tabA"
            )
            t2_tabB = dram.tile(
                [(pl.R2 - pl.R1) // 2, D2], bf16, addr_space="Shared",
                name="t2_tabB",
            )
            t2_tabC = dram.tile(
                [(pl.n_pad - pl.R2) // 2, D2], bf16, addr_space="Shared",
                name="t2_tabC",
            )

            w1_sb = cpool.tile([f_in, hid], bf16)
            nc.sync.dma_start(w1_sb[:], w1_in[:])
            w2_sb = cpool.tile([hid, cls_], bf16)
            nc.sync.dma_start(w2_sb[:], w2_in[:])
            b1_sb = cpool.tile([P, hid], fp32)
            nc.sync.dma_start(b1_sb[:], b1_in[:])
            b2_sb = cpool.tile([P, cls_], fp32)
            nc.sync.dma_start(b2_sb[:], b2_in[:])
            eye_sb = cpool.tile([P, P], fp32)
            nc.sync.dma_start(eye_sb[:], eye_in[:])
            iota_sb = cpool.tile([P, 2 * P], bf16)
            nc.sync.dma_start(iota_sb[:], iota_in[:])
            dv_sb = cpool.tile([P, n_win], fp32)
            nc.sync.dma_start(dv_sb[:], dv_in[:])
            dv2_sb = cpool.tile([P, n_win], fp32)
            nc.sync.dma_start(dv2_sb[:], dv2_in[:])
            idx_sb = cpool.tile([P, pl.total_slots // 16], i16)
            nc.sync.dma_start(idx_sb[:], idx_in[:])
            dr_sb = cpool.tile([P, pl.total_chunks], fp32)
            nc.sync.dma_start(dr_sb[:], dr_in[:])
            own1_all = cpool.tile([P, n_win, hid], bf16)
            own2_all = cpool.tile([P, n_win, hid], bf16)
            accA = cpool.tile([P, n_win, hid], bf16)
            obr_all = cpool.tile([P, n_win, cls_], fp32)
            se_all = cpool.tile([P, n_win], fp32)
            ls_all = cpool.tile([P, n_win], fp32)

            # ---- prep: replicated full table (core order) + own terms ----
            # dinv is folded into x host-side, so the table cast is a plain
            # batched psum->bf16 copy and own1 = dinv*Hs + b1.
            PB = 7  # windows per psum tile / cast op
            for g0 in range(0, NWG, G_WIN):
                gn = min(G_WIN, NWG - g0)
                xT = prep.tile([P, gn, P], bf16, tag="xT")
                nc.sync.dma_start(xT[:], xt_in[:, g0 : g0 + gn, :])
                hsd = prep.tile([P, gn, hid], bf16, tag="hsd")
                for k0 in range(0, gn, PB):
                    kb = min(PB, gn - k0)
                    ph4 = prep_ps.tile([P, PB, hid], fp32, tag="ph")
                    for j in range(kb):
                        w = g0 + k0 + j
                        nc.tensor.matmul(
                            ph4[:, j, :],
                            xT[:, k0 + j, :],
                            w1_sb[:],
                            start=True,
                            stop=True,
                        )
                        if w < n_win:  # own window (core order => first 98)
                            nc.vector.scalar_tensor_tensor(
                                own1_all[:, w, :],
                                ph4[:, j, :],
                                dv_sb[:, w : w + 1],
                                b1_sb[:],
                                Alu.mult,
                                Alu.add,
                            )
                    nc.scalar.activation(
                        hsd[:, k0 : k0 + kb, :], ph4[:, :kb, :], Act.Copy
                    )
                nc.sync.dma_start(hs_tab[g0 // G_WIN, :, :, :], hsd[:])

            def hs_view(s):
                pa, pb = l1_pair[s]
                return _AP(
                    hs_tab[:, :, :, :].tensor, pa * D2, [[D2, pb - pa], [1, D2]]
                )

            def gathers(streams, views):
                mt = {}
                for s, blk, c0, ck, _w in _call_list(pl, streams):
                    m = msgs_pool.tile([P, ck, D2], bf16, tag="m")
                    gc0 = int(pl.sstart[s]) + c0
                    nc.gpsimd.dma_gather(
                        m[:],
                        views[s - streams[0]],
                        idx_sb[:, gc0 * 8 : (gc0 + ck) * 8],
                        ck * P,
                        ck * P,
                        D2,
                    )
                    mt[(s, blk)] = m
                return mt

            def window_agg(pw, w, streams, mt, preloaded=False):
                nmm = 1 if preloaded else 0
                tot = 2 * int(sum(pl.cap[s][w] for s in streams)) + nmm
                for s in streams:
                    capw = int(pl.cap[s][w])
                    for k in range(capw):
                        pos = int(pl.woff[s][w]) + k
                        col = int(pl.sstart[s]) + pos
                        blk, off = divmod(pos, CC)
                        S = oneh.tile([P, 2 * P], bf16, tag="S")
                        nc.vector.tensor_scalar(
                            S[:],
                            iota_sb[:],
                            dr_sb[:, col : col + 1],
                            None,
                            Alu.is_equal,
                        )
                        for half in range(2):
                            nc.tensor.matmul(
                                pw[:],
                                S[:, half * P : (half + 1) * P],
                                mt[(s, blk)][
                                    :, off, half * hid : (half + 1) * hid
                                ],
                                start=(nmm == 0),
                                stop=(nmm == tot - 1),
                            )
                            nmm += 1

            # ---- layer 1 emit --------------------------------------------
            mt1 = gathers([0, 1], [hs_view(0), hs_view(1)])
            NW1, NW2 = pl.HS1 // P, pl.HS2 // P  # 42, 28
            ag_pieces = {
                2: (t2_shard[0:NW1, :, :], t2_tabA),
                4: (t2_shard[NW1 : NW1 + NW2, :, :], t2_tabB),
                6: (t2_shard[NW1 + NW2 : n_win, :, :], t2_tabC),
            }
            for gi, ws in enumerate(groups):
                g0, gn = ws[0], len(ws)
                t2g = epi.tile([P, gn, hid], bf16, tag="t2g")
                for wi, w in enumerate(ws):
                    pw = agg_ps.tile([P, hid], fp32, tag="agg")
                    window_agg(pw, w, [0, 1], mt1)
                    u = epi.tile([P, hid], fp32, tag="u")
                    nc.vector.scalar_tensor_tensor(
                        u[:],
                        pw[:],
                        dv_sb[:, w : w + 1],
                        own1_all[:, w, :],
                        Alu.mult,
                        Alu.add,
                    )
                    nc.scalar.activation(
                        t2g[:, wi, :], u[:], Act.Relu, scale=dv_sb[:, w : w + 1]
                    )
                    nc.scalar.activation(
                        own2_all[:, w, :], u[:], Act.Relu,
                        scale=dv2_sb[:, w : w + 1],
                    )
                nc.sync.dma_start(
                    t2_shard[g0 : g0 + gn, :, :].transpose([1, 0, 2]), t2g[:]
                )
                if gi in ag_pieces:
                    shard_in, tab_out = ag_pieces[gi]
                    nc.gpsimd.collective_compute(
                        "AllGather",
                        Alu.bypass,
                        replica_groups=[list(range(pl.n_cores))],
                        ins=[shard_in.opt()],
                        outs=[tab_out[:, :].opt()],
                    )

            # ---- layer 2 phase A (after AG2a) ----------------------------
            mtA = gathers([2], [t2_tabA[:, :]])
            for w in range(n_win):
                pw = agg_ps.tile([P, hid], fp32, tag="agg")
                window_agg(pw, w, [2], mtA)
                nc.scalar.activation(accA[:, w, :], pw[:], Act.Copy)

            # ---- layer 2 phase B (after AG2b): accumulate into accA ------
            mtB = gathers([3], [t2_tabB[:, :]])
            for w in range(n_win):
                pw = agg_ps.tile([P, hid], fp32, tag="agg")
                window_agg(pw, w, [3], mtB)
                nc.vector.tensor_tensor(
                    out=accA[:, w, :], in0=pw[:], in1=accA[:, w, :], op=Alu.add
                )

            # ---- layer 2 phase C + head ----------------------------------
            mtC = gathers([4], [t2_tabC[:, :]])
            for w in range(n_win):
                pw = agg_ps.tile([P, hid], fp32, tag="agg")
                nc.scalar.activation(pw[:], accA[:, w, :], Act.Copy)
                window_agg(pw, w, [4], mtC, preloaded=True)
                u = epi.tile([P, hid], fp32, tag="u")
                nc.vector.scalar_tensor_tensor(
                    u[:],
                    pw[:],
                    dv_sb[:, w : w + 1],
                    own2_all[:, w, :],
                    Alu.mult,
                    Alu.add,
                )
                ztp = fin_ps.tile([hid, P], fp32, tag="ztp")
                nc.tensor.transpose(ztp[:], u[:], eye_sb[:])
                zt = epi.tile([hid, P], bf16, tag="zt")
                nc.scalar.activation(zt[:], ztp[:], Act.Copy)
                ops = fin2_ps.tile([P, cls_], fp32, tag="ops")
                nc.tensor.matmul(ops[:], zt[:], w2_sb[:], start=True, stop=True)
                ob = epi.tile([P, cls_], fp32, tag="ob")
                nc.vector.tensor_tensor(
                    out=ob[:], in0=ops[:], in1=b2_sb[:], op=Alu.add
                )
                obr = obr_all[:, w, :]
                nc.scalar.activation(obr, ob[:], Act.Relu)
                ex = epi.tile([P, cls_], fp32, tag="ex")
                nc.scalar.activation(
                    ex[:], obr, Act.Exp, accum_out=se_all[:, w : w + 1]
                )

            # log_softmax tail (logits tiny; no max-subtraction needed)
            nc.scalar.activation(ls_all[:], se_all[:], Act.Ln)
            ostage = cpool.tile([P, n_win, cls_], fp32)
            nc.vector.tensor_tensor(
                out=ostage[:],
                in0=obr_all[:],
                in1=ls_all[:].unsqueeze(2).to_broadcast([P, n_win, cls_]),
                op=Alu.subtract,
            )
            nc.sync.dma_start(out_t[:], ostage[:])

    nc.compile()
    return nc


# ----------------------------------------------------------------------------
# inputs / entry point
# ----------------------------------------------------------------------------
def make_in_maps(pl, x, W1, b1, W2, b2, f_in=F_IN):
    import ml_dtypes

    bf16 = ml_dtypes.bfloat16
    n = x.shape[0]
    x2 = np.zeros((pl.n_pad, f_in), dtype=np.float32)
    x2[pl.newrow[:n]] = np.asarray(x, dtype=np.float32)
    x2 *= pl.dinv[:, None]  # fold dinv into x: Hs = (dinv*x) @ W1
    x2w = x2.reshape(pl.NWG, P, f_in)
    shared = {
        "w1": np.ascontiguousarray(W1).astype(bf16),
        "w2": np.ascontiguousarray(W2).astype(bf16),
        "b1r": np.tile(np.asarray(b1, dtype=np.float32), (P, 1)),
        "b2r": np.tile(np.asarray(b2, dtype=np.float32), (P, 1)),
        "eye": np.eye(P, dtype=np.float32),
        "iota": np.tile(np.arange(2 * P, dtype=np.float32), (P, 1)).astype(
            bf16
        ),
    }
    maps = []
    for c in range(pl.n_cores):
        xt = np.ascontiguousarray(x2w[pl.corder[c]].transpose(2, 0, 1)).astype(
            bf16
        )
        maps.append(
            dict(
                shared,
                xt=xt,
                idx=pl.idx16[c],
                dr=pl.drel[c],
                dv=pl.dv[c],
                dv2=pl.dv2[c],
            )
        )
    return maps


_LAST_NC = None


def kernel(x, edge_index, W1, b1, W2, b2):
    global _EXEC_NS, _LAST_NC
    from concourse.bass_utils import run_bass_kernel_spmd

    x = np.asarray(x)
    src = np.asarray(edge_index[0]).astype(np.int64)
    dst = np.asarray(edge_index[1]).astype(np.int64)
    n = x.shape[0]

    pl = make_plan(src, dst, n)
    nc = build_nc(pl)
    _LAST_NC = nc
    in_maps = make_in_maps(pl, x, W1, b1, W2, b2)

    res = run_bass_kernel_spmd(nc, in_maps, core_ids=list(range(pl.n_cores)))
    _EXEC_NS = res.exec_time_ns
    full = np.empty((pl.n_pad, CLS), dtype=np.float32)
    for c in range(pl.n_cores):
        o = np.asarray(res.results[c]["out"])  # [P, n_win, CLS]
        full[c * pl.npc : (c + 1) * pl.npc] = o.transpose(1, 0, 2).reshape(
            pl.npc, CLS
        )
    return full[pl.newrow[:n]].astype(np.float32)
